# revision 15
# baseline (speedup 1.0000x reference)
"""LinearKAN Trainium2 kernel — bf16 central planes + fp8 DoubleRow edge planes.

Math (per reference):
    phi[b,a,i] = exp(-((x[b,i] - g_a)/h)^2)     g = linspace(-2, 2, 8), h = 4/7
    out[b,o]   = sum_{a,i} phi[b,a,i]*(c[a,o,i]*w_s[o,i]) + sum_i silu(x[b,i])*w_b[o,i]

Precision split: the two EDGE grid planes (a=0, a=7) carry only ~7% of the
phi energy under x~N(0,1), so quantizing THEM (and their weights) to fp8-e4m3
costs ~1.2e-2 relative error total (gate 2e-2) while letting them run as ONE
fp8 DoubleRow matmul per i-tile at 0.5 cycles/row — removing 12 of 48 bf16
k-tiles (~25% of PE work) for ~3% of it.

Structure per core (batch-sharded, B_SHARD=2048 over 4 b-tiles of 512):
  - bf16 part: 36 k-tiles (it x a=1..6), k-major, 6 psum banks (o=0..5),
    phi via one ACT op each (Derivative_Erf = 2/sqrt(pi) exp(-z^2), the
    sqrt(pi)/2 folded into W).
  - fp8 part: per (bt, it) a pair tile [128, 2, 512] f8e4 holds (phi_a0,
    phi_a7); weights [128, 2, 768] f8e4 scaled by 256. Per (bt, o) the 6
    DR matmuls accumulate in a SEPARATE psum bank; result is combined as
    ot += psB * 2^-8 on the DVE at drain.
  - PSUM budget forces the DR clusters of b-tile N to run INSIDE b-tile
    N+1's k-loop (6 bf16-psums + 2 rotating DR-psums = 8 banks). The last
    b-tile is o-major, hosting bt2's clusters and its own per pass.
  - silu residual folded into ALL planes via the N(0,1)-weighted lsq fit
    silu ~= sum_a BETA[a] phi_a.

Schedule notes (measured on HW):
  - DR matmuls pace at ~215ns (1024 fp8 cols at 2/cycle) when CLUSTERED;
    alternating bf16<->DR costs ~190ns mode-switch each way, so clusters
    of 6 beat 1:1 interleaving.
  - x is bf16 end-to-end (+5e-4 rel err): halves x DMA traffic so the
    sync queue can feed W at the warm MM stream's 151 GB/s from t~12us.
  - all x tiles issue in the first ~35us (late x issues head-of-line
    block the ACT queue via scheduler-hoisted future-bt phi ACTIVATEs).
  - 11-matmul warmup train on gpsimd-memset tiles bridges the PE until
    the real stream so the HAM clock-gate opens (2.4GHz) before it.
  - out tiles DMA via the gpsimd queue (256KB f32 tiles on sync would
    delay the next octaves' x/W arrivals at b-tile boundaries).
"""

import ml_dtypes
import numpy as np

import concourse.bacc as bacc
import concourse.tile as tile
from concourse import mybir
from concourse.bass_utils import run_bass_kernel_spmd

N_CORES = 8
BATCH, IN_F, OUT_F = 16384, 768, 768
B_SHARD = BATCH // N_CORES          # 2048
GRID_SIZE, GRID_LO, GRID_HI = 8, -2.0, 2.0
H = (GRID_HI - GRID_LO) / (GRID_SIZE - 1)
P = 128
I_TILES = IN_F // P                 # 6
O_TILES = OUT_F // P                # 6
A_BF = list(range(1, 7))            # bf16 planes
A_F8 = (0, 7)                       # fp8 DoubleRow pair
K_BF = I_TILES * len(A_BF)          # 36 bf16 k-tiles
B_TILE = 512
N_BTILES = B_SHARD // B_TILE        # 4
W8_SCALE = 256.0

F32 = mybir.dt.float32
BF16 = mybir.dt.bfloat16
F8 = mybir.dt.float8e4
AF = mybir.ActivationFunctionType
DR = mybir.MatmulPerfMode.DoubleRow
ADD = mybir.AluOpType.add
SP2 = float(np.sqrt(np.pi) / 2.0)

N_WARMUP_MM = 11


def _silu_fit():
    X = np.linspace(-5.6, 5.6, 4481)
    W = np.exp(-X * X / 2.0)
    SW = np.sqrt(W / W.sum())
    grid = np.linspace(GRID_LO, GRID_HI, GRID_SIZE)
    cols = [np.exp(-(((X - g) / H) ** 2)) for g in grid]
    A = (np.array(cols) * SW[None, :]).T
    b = (X / (1.0 + np.exp(-X))) * SW
    coef, *_ = np.linalg.lstsq(A, b, rcond=None)
    return [float(v) for v in coef]


BETA = _silu_fit()


def _build_nc():
    nc = bacc.Bacc(None, target_bir_lowering=False, debug=False)

    xT = nc.dram_tensor("xT", [IN_F, B_SHARD], BF16, kind="ExternalInput")
    wT = nc.dram_tensor("wT", [K_BF, P, OUT_F], BF16, kind="ExternalInput")
    wT8 = nc.dram_tensor("wT8", [I_TILES, P, 2, OUT_F], F8,
                         kind="ExternalInput")
    outT = nc.dram_tensor("outT", [OUT_F, B_SHARD], F32, kind="ExternalOutput")

    xT_ap = xT.ap()
    wT_ap = wT.ap()
    wT8_ap = wT8.ap()
    outT_ap = outT.ap()

    grid = np.linspace(GRID_LO, GRID_HI, GRID_SIZE, dtype=np.float64)

    with tile.TileContext(nc) as tc:
        with (
            tc.tile_pool(name="wpool", bufs=1) as wpool,
            tc.tile_pool(name="misc", bufs=1) as misc,
            tc.tile_pool(name="xpool", bufs=24) as xpool,
            tc.tile_pool(name="phipool", bufs=20) as phipool,
            tc.tile_pool(name="pairpool", bufs=14) as pairpool,
            tc.tile_pool(name="phi3pool", bufs=1) as phi3pool,
            tc.tile_pool(name="opool", bufs=12) as opool,
            tc.tile_pool(name="tmppool", bufs=3) as tmppool,
            tc.tile_pool(name="psum", bufs=6, space="PSUM") as psum_pool,
            tc.tile_pool(name="psumB", bufs=2, space="PSUM") as psumB_pool,
        ):
            # ---- PE warmup tiles first: gpsimd memsets run earliest
            wa = misc.tile([P, P], BF16, tag="warm_a", name="warm_a")
            nc.gpsimd.memset(wa, 0.0)
            wb_ = misc.tile([P, B_TILE], BF16, tag="warm_b", name="warm_b")
            nc.gpsimd.memset(wb_, 0.0)

            # ---- x00 split into partition-row quarters across 4 DMA queues
            x_tiles = {}
            xt = xpool.tile([P, B_TILE], BF16, tag="x", name="x0_0")
            qp = P // 4
            qeng = [nc.sync, nc.scalar, nc.scalar, nc.gpsimd]
            for q in range(4):
                qeng[q].dma_start(out=xt[q * qp:(q + 1) * qp, :],
                                  in_=xT_ap[q * qp:(q + 1) * qp, 0:B_TILE])
            x_tiles[(0, 0)] = xt

            w_tiles = [None] * K_BF

            def w_load(k):
                wt = wpool.tile([P, OUT_F], BF16, tag=f"w{k}", name=f"w{k}")
                nc.sync.dma_start(out=wt, in_=wT_ap[k])
                w_tiles[k] = wt

            def x_load(bt, it):
                bsl0 = slice(bt * B_TILE, (bt + 1) * B_TILE)
                xt2 = xpool.tile([P, B_TILE], BF16, tag="x", name=f"x{bt}_{it}")
                nc.sync.dma_start(out=xt2,
                                  in_=xT_ap[it * P:(it + 1) * P, bsl0])
                x_tiles[(bt, it)] = xt2

            # sync FIFO: W back-to-back with x tiles threaded in early enough
            # that every x lands by ~35us (late x issues head-of-line block
            # the ACT queue via scheduler-hoisted future-bt phi ACTIVATEs)
            kq = 0
            for it2 in range(1, I_TILES):
                w_load(kq); w_load(kq + 1)
                kq += 2
                x_load(0, it2)
            rest_x = [(bt, it) for bt in range(1, N_BTILES)
                      for it in range(I_TILES)]
            for bt2, it2 in rest_x:
                if kq < K_BF:
                    w_load(kq)
                    kq += 1
                x_load(bt2, it2)
            for k in range(kq, K_BF):
                w_load(k)
            # fp8 weight pairs: consumed from bt1 onward (~75us) — load last
            w8_tiles = []
            for it2 in range(I_TILES):
                w8 = wpool.tile([P, 2, OUT_F], F8, tag=f"w8_{it2}",
                                name=f"w8_{it2}")
                nc.sync.dma_start(out=w8, in_=wT8_ap[it2])
                w8_tiles.append(w8)

            # ---- per-a bias tiles for Derivative_Erf: -g_a/h ----
            bias_tiles = []
            for a in range(GRID_SIZE):
                bt_ = misc.tile([P, 1], F32, tag=f"bias{a}", name=f"bias{a}")
                nc.vector.memset(bt_, float(-grid[a] / H))
                bias_tiles.append(bt_)

            # dummy activation: hoists the one-time ACT_TABLE_LOAD early
            scr = misc.tile([P, 1], F32, tag="scr", name="scr")
            nc.scalar.activation(out=scr, in_=bias_tiles[0],
                                 func=AF.Derivative_Erf,
                                 bias=bias_tiles[0], scale=1.0 / H)

            # ---- PE warmup train (HAM clock-gate): bridges the PE until the
            # real stream starts so HAM un-throttles before real MMs run
            wp = psumB_pool.tile([P, B_TILE], F32, tag="psB", name="warm_ps")
            for i in range(N_WARMUP_MM):
                nc.tensor.matmul(wp, wa, wb_, start=(i == 0),
                                 stop=(i == N_WARMUP_MM - 1))

            ot_tiles = {}
            pair_tiles = {}

            def host_cluster(btc, o):
                """Run btc's 6 DR matmuls for o-block o, combine into its out
                tile, and DMA it. Runs inside b-tile btc+1's window."""
                osl = slice(o * P, (o + 1) * P)
                psB = psumB_pool.tile([P, B_TILE], F32, tag="psB",
                                      name=f"psB{btc}_{o}")
                for itp in range(I_TILES):
                    nc.tensor.matmul(
                        psB,
                        w8_tiles[itp][:, :, osl],
                        pair_tiles[(btc, itp)],
                        start=(itp == 0),
                        stop=(itp == I_TILES - 1),
                        perf_mode=DR,
                    )
                tmp = tmppool.tile([P, B_TILE], F32, tag="tmp",
                                   name=f"tmp{btc}_{o}")
                nc.vector.tensor_scalar_mul(tmp, psB, 1.0 / W8_SCALE)
                ot = ot_tiles[(btc, o)]
                nc.vector.tensor_tensor(ot, ot, tmp, ADD)
                bsl0 = slice(btc * B_TILE, (btc + 1) * B_TILE)
                nc.gpsimd.dma_start(out=outT_ap[osl, bsl0], in_=ot)

            # ---- main loop ----
            for bt in range(N_BTILES):
                bsl = slice(bt * B_TILE, (bt + 1) * B_TILE)
                last = bt == N_BTILES - 1

                # phi production: 6 bf16 planes + 1 fp8 pair tile per octave
                phis = []
                for it in range(I_TILES):
                    for a in A_BF:
                        if last:
                            ph = phi3pool.tile([P, B_TILE], BF16,
                                               tag=f"phi3_{it}_{a}",
                                               name=f"phi3_{it}_{a}")
                        else:
                            ph = phipool.tile([P, B_TILE], BF16, tag="phi",
                                              name=f"phi{bt}_{it}_{a}")
                        nc.scalar.activation(out=ph, in_=x_tiles[(bt, it)],
                                             func=AF.Derivative_Erf,
                                             bias=bias_tiles[a], scale=1.0 / H)
                        phis.append(ph)
                    if last:
                        pr = phi3pool.tile([P, 2, B_TILE], F8,
                                           tag=f"pair3_{it}",
                                           name=f"pair3_{it}")
                    else:
                        pr = pairpool.tile([P, 2, B_TILE], F8, tag="pair",
                                           name=f"pair{bt}_{it}")
                    for j, a in enumerate(A_F8):
                        nc.scalar.activation(out=pr[:, j, :],
                                             in_=x_tiles[(bt, it)],
                                             func=AF.Derivative_Erf,
                                             bias=bias_tiles[a], scale=1.0 / H)
                    pair_tiles[(bt, it)] = pr

                if not last:
                    psums = []
                    for o in range(O_TILES):
                        ps = psum_pool.tile([P, B_TILE], F32, tag="ps",
                                            name=f"ps{bt}_{o}")
                        psums.append(ps)
                    for k in range(K_BF):
                        it = k // len(A_BF)
                        for o in range(O_TILES):
                            nc.tensor.matmul(
                                psums[o],
                                w_tiles[k][:, o * P:(o + 1) * P],
                                phis[k],
                                start=(k == 0),
                                stop=(k == K_BF - 1),
                            )
                        # host the previous b-tile's DR cluster for o=it
                        # right after octave it's bf16 matmuls
                        if bt > 0 and k % len(A_BF) == len(A_BF) - 1:
                            host_cluster(bt - 1, it)
                    for o in range(O_TILES):
                        ot = opool.tile([P, B_TILE], F32, tag="out",
                                        name=f"out{bt}_{o}")
                        nc.vector.tensor_copy(ot, psums[o])
                        ot_tiles[(bt, o)] = ot
                else:
                    # o-major passes; each hosts bt2's DR cluster + its own
                    for o in range(O_TILES):
                        osl = slice(o * P, (o + 1) * P)
                        host_cluster(N_BTILES - 2, o)

                        psB3 = psumB_pool.tile([P, B_TILE], F32, tag="psB",
                                               name=f"psB3_{o}")
                        for itp in range(I_TILES):
                            nc.tensor.matmul(
                                psB3,
                                w8_tiles[itp][:, :, osl],
                                pair_tiles[(bt, itp)],
                                start=(itp == 0),
                                stop=(itp == I_TILES - 1),
                                perf_mode=DR,
                            )

                        ps = psum_pool.tile([P, B_TILE], F32, tag="ps",
                                            name=f"ps3_{o}")
                        for k in range(K_BF):
                            nc.tensor.matmul(
                                ps,
                                w_tiles[k][:, osl],
                                phis[k],
                                start=(k == 0),
                                stop=(k == K_BF - 1),
                            )
                        ot = opool.tile([P, B_TILE], F32, tag="out",
                                        name=f"out3_{o}")
                        nc.vector.tensor_copy(ot, ps)
                        tmp = tmppool.tile([P, B_TILE], F32, tag="tmp",
                                           name=f"tmp3_{o}")
                        nc.vector.tensor_scalar_mul(tmp, psB3, 1.0 / W8_SCALE)
                        nc.vector.tensor_tensor(ot, ot, tmp, ADD)
                        if o == O_TILES - 1:
                            # final tile: split the out DMA across two queues
                            hb = B_TILE // 2
                            b0 = bt * B_TILE
                            engs = [nc.sync, nc.scalar]
                            for half in range(2):
                                engs[half].dma_start(
                                    out=outT_ap[osl,
                                                b0 + half * hb:
                                                b0 + (half + 1) * hb],
                                    in_=ot[:, half * hb:(half + 1) * hb])
                        else:
                            nc.sync.dma_start(out=outT_ap[osl, bsl], in_=ot)

    nc.compile()
    return nc


_NC_CACHE = {}


def _get_nc():
    if "nc" not in _NC_CACHE:
        _NC_CACHE["nc"] = _build_nc()
    return _NC_CACHE["nc"]


def _fold_weights(c, w_s, w_b):
    """Host fold: cw[a,o,i] = SP2*(c*w_s + BETA[a]*w_b); bf16 central planes
    as [K_BF, P, OUT_F], fp8 edge pairs as [I_TILES, P, 2, OUT_F]*256."""
    beta = np.asarray(BETA, dtype=np.float32)[:, None, None]
    cw = SP2 * (c * w_s[None, :, :] + beta * w_b[None, :, :])  # [a, o, i]
    W = np.ascontiguousarray(cw.transpose(0, 2, 1))            # [a, i, o]
    Wk = np.empty((K_BF, P, OUT_F), np.float32)
    for it in range(I_TILES):
        for ja, a in enumerate(A_BF):
            Wk[it * len(A_BF) + ja] = W[a, it * P:(it + 1) * P, :]
    W8 = np.empty((I_TILES, P, 2, OUT_F), np.float32)
    for it in range(I_TILES):
        for j, a in enumerate(A_F8):
            W8[it, :, j, :] = W[a, it * P:(it + 1) * P, :] * W8_SCALE
    return (Wk.astype(ml_dtypes.bfloat16),
            W8.astype(ml_dtypes.float8_e4m3))


def kernel(x, w_b, w_s, c):
    x = np.ascontiguousarray(np.asarray(x, dtype=np.float32))
    w_b = np.asarray(w_b, dtype=np.float32)
    w_s = np.asarray(w_s, dtype=np.float32)
    c = np.asarray(c, dtype=np.float32)

    xT = np.ascontiguousarray(x.T).astype(ml_dtypes.bfloat16)  # [IN_F, BATCH]
    wT, wT8 = _fold_weights(c, w_s, w_b)

    in_maps = []
    for ci in range(N_CORES):
        in_maps.append({
            "xT": np.ascontiguousarray(xT[:, ci * B_SHARD:(ci + 1) * B_SHARD]),
            "wT": wT,
            "wT8": wT8,
        })

    res = run_bass_kernel_spmd(_get_nc(), in_maps, core_ids=list(range(N_CORES)))
    outT = np.concatenate([r["outT"] for r in res.results], axis=1)
    return np.ascontiguousarray(outT.T).astype(np.float32, copy=False)


if __name__ == "__main__":
    rng = np.random.default_rng(0)
    x = rng.standard_normal((BATCH, IN_F), dtype=np.float32)
    w_b = rng.standard_normal((OUT_F, IN_F), dtype=np.float32) * 1e-3
    w_s = np.ones((OUT_F, IN_F), dtype=np.float32)
    c = (rng.standard_normal((GRID_SIZE, OUT_F, IN_F)) * 1e-3).astype(np.float32)
    out = kernel(x, w_b, w_s, c)
    print(out.shape, out.dtype)


# revision 16
# speedup vs baseline: 1.0791x; 1.0791x over previous
"""LinearKAN Trainium2 kernel — bf16 central planes + fp8 DoubleRow edge planes.

Math (per reference):
    phi[b,a,i] = exp(-((x[b,i] - g_a)/h)^2)     g = linspace(-2, 2, 8), h = 4/7
    out[b,o]   = sum_{a,i} phi[b,a,i]*(c[a,o,i]*w_s[o,i]) + sum_i silu(x[b,i])*w_b[o,i]

Precision split: the two EDGE grid planes (a=0, a=7) carry only ~7% of the
phi energy under x~N(0,1), so quantizing THEM (and their weights) to fp8-e4m3
costs ~1.2e-2 relative error total (gate 2e-2) while letting them run as ONE
fp8 DoubleRow matmul per i-tile at 0.5 cycles/row — removing 12 of 48 bf16
k-tiles (~25% of PE work) for ~3% of it.

Structure per core (batch-sharded, B_SHARD=2048 over 4 b-tiles of 512):
  - bf16 part: 36 k-tiles (it x a=1..6), k-major, 6 psum banks (o=0..5),
    phi via one ACT op each (Derivative_Erf = 2/sqrt(pi) exp(-z^2), the
    sqrt(pi)/2 folded into W).
  - fp8 part: per (bt, it) a pair tile [128, 2, 512] f8e4 holds (phi_a0,
    phi_a7); weights [128, 2, 768] f8e4 scaled by 256. Per (bt, o) the 6
    DR matmuls accumulate in a SEPARATE psum bank; result is combined as
    ot += psB * 2^-8 on the DVE at drain.
  - PSUM budget forces the DR clusters of b-tile N to run INSIDE b-tile
    N+1's k-loop (6 bf16-psums + 2 rotating DR-psums = 8 banks). The last
    b-tile is o-major, hosting bt2's clusters and its own per pass.
  - silu residual folded into ALL planes via the N(0,1)-weighted lsq fit
    silu ~= sum_a BETA[a] phi_a.

Schedule notes (measured on HW):
  - DR matmuls pace at ~215ns (1024 fp8 cols at 2/cycle) when CLUSTERED;
    alternating bf16<->DR costs ~190ns mode-switch each way, so clusters
    of 6 beat 1:1 interleaving.
  - x is bf16 end-to-end (+5e-4 rel err): halves x DMA traffic so the
    sync queue can feed W at the warm MM stream's 151 GB/s from t~12us.
  - all x tiles issue in the first ~35us (late x issues head-of-line
    block the ACT queue via scheduler-hoisted future-bt phi ACTIVATEs).
  - 11-matmul warmup train on gpsimd-memset tiles bridges the PE until
    the real stream so the HAM clock-gate opens (2.4GHz) before it.
  - out tiles DMA via the gpsimd queue (256KB f32 tiles on sync would
    delay the next octaves' x/W arrivals at b-tile boundaries).
"""

import ml_dtypes
import numpy as np

import concourse.bacc as bacc
import concourse.tile as tile
from concourse import mybir
from concourse.bass_utils import run_bass_kernel_spmd

N_CORES = 8
BATCH, IN_F, OUT_F = 16384, 768, 768
B_SHARD = BATCH // N_CORES          # 2048
GRID_SIZE, GRID_LO, GRID_HI = 8, -2.0, 2.0
H = (GRID_HI - GRID_LO) / (GRID_SIZE - 1)
P = 128
I_TILES = IN_F // P                 # 6
O_TILES = OUT_F // P                # 6
A_BF = list(range(1, 7))            # bf16 planes
A_F8 = (0, 7)                       # fp8 DoubleRow pair
K_BF = I_TILES * len(A_BF)          # 36 bf16 k-tiles
B_TILE = 512
N_BTILES = B_SHARD // B_TILE        # 4
W8_SCALE = 256.0

F32 = mybir.dt.float32
BF16 = mybir.dt.bfloat16
F8 = mybir.dt.float8e4
AF = mybir.ActivationFunctionType
DR = mybir.MatmulPerfMode.DoubleRow
ADD = mybir.AluOpType.add
SP2 = float(np.sqrt(np.pi) / 2.0)

N_WARMUP_MM = 11


def _silu_fit():
    X = np.linspace(-5.6, 5.6, 4481)
    W = np.exp(-X * X / 2.0)
    SW = np.sqrt(W / W.sum())
    grid = np.linspace(GRID_LO, GRID_HI, GRID_SIZE)
    cols = [np.exp(-(((X - g) / H) ** 2)) for g in grid]
    A = (np.array(cols) * SW[None, :]).T
    b = (X / (1.0 + np.exp(-X))) * SW
    coef, *_ = np.linalg.lstsq(A, b, rcond=None)
    return [float(v) for v in coef]


BETA = _silu_fit()


def _build_nc():
    nc = bacc.Bacc(None, target_bir_lowering=False, debug=False)

    xT = nc.dram_tensor("xT", [IN_F, B_SHARD], BF16, kind="ExternalInput")
    wT = nc.dram_tensor("wT", [K_BF, P, OUT_F], BF16, kind="ExternalInput")
    wT8 = nc.dram_tensor("wT8", [I_TILES, P, 2, OUT_F], F8,
                         kind="ExternalInput")
    outT = nc.dram_tensor("outT", [OUT_F, B_SHARD], F32, kind="ExternalOutput")

    xT_ap = xT.ap()
    wT_ap = wT.ap()
    wT8_ap = wT8.ap()
    outT_ap = outT.ap()

    grid = np.linspace(GRID_LO, GRID_HI, GRID_SIZE, dtype=np.float64)

    with tile.TileContext(nc) as tc:
        with (
            tc.tile_pool(name="wpool", bufs=1) as wpool,
            tc.tile_pool(name="misc", bufs=1) as misc,
            tc.tile_pool(name="xpool", bufs=24) as xpool,
            tc.tile_pool(name="phipool", bufs=20) as phipool,
            tc.tile_pool(name="pairpool", bufs=14) as pairpool,
            tc.tile_pool(name="phi3pool", bufs=1) as phi3pool,
            tc.tile_pool(name="opool", bufs=12) as opool,
            tc.tile_pool(name="tmppool", bufs=3) as tmppool,
            tc.tile_pool(name="psum", bufs=6, space="PSUM") as psum_pool,
            tc.tile_pool(name="psumB", bufs=2, space="PSUM") as psumB_pool,
        ):
            # ---- PE warmup tiles first: gpsimd memsets run earliest
            wa = misc.tile([P, P], BF16, tag="warm_a", name="warm_a")
            nc.gpsimd.memset(wa, 0.0)
            wb_ = misc.tile([P, B_TILE], BF16, tag="warm_b", name="warm_b")
            nc.gpsimd.memset(wb_, 0.0)

            # ---- x00 split into partition-row quarters across 4 DMA queues
            x_tiles = {}
            xt = xpool.tile([P, B_TILE], BF16, tag="x", name="x0_0")
            qp = P // 4
            qeng = [nc.sync, nc.scalar, nc.scalar, nc.gpsimd]
            for q in range(4):
                qeng[q].dma_start(out=xt[q * qp:(q + 1) * qp, :],
                                  in_=xT_ap[q * qp:(q + 1) * qp, 0:B_TILE])
            x_tiles[(0, 0)] = xt

            w_tiles = [None] * K_BF

            def w_load(k):
                wt = wpool.tile([P, OUT_F], BF16, tag=f"w{k}", name=f"w{k}")
                nc.sync.dma_start(out=wt, in_=wT_ap[k])
                w_tiles[k] = wt

            def x_load(bt, it):
                bsl0 = slice(bt * B_TILE, (bt + 1) * B_TILE)
                xt2 = xpool.tile([P, B_TILE], BF16, tag="x", name=f"x{bt}_{it}")
                nc.sync.dma_start(out=xt2,
                                  in_=xT_ap[it * P:(it + 1) * P, bsl0])
                x_tiles[(bt, it)] = xt2

            # sync FIFO: W back-to-back with x tiles threaded in early enough
            # that every x lands by ~35us (late x issues head-of-line block
            # the ACT queue via scheduler-hoisted future-bt phi ACTIVATEs)
            kq = 0
            for it2 in range(1, I_TILES):
                w_load(kq); w_load(kq + 1)
                kq += 2
                x_load(0, it2)
            rest_x = [(bt, it) for bt in range(1, N_BTILES)
                      for it in range(I_TILES)]
            for bt2, it2 in rest_x:
                if kq < K_BF:
                    w_load(kq)
                    kq += 1
                x_load(bt2, it2)
            for k in range(kq, K_BF):
                w_load(k)
            # fp8 weight pairs: consumed from bt1 onward (~75us) — load last
            w8_tiles = []
            for it2 in range(I_TILES):
                w8 = wpool.tile([P, 2, OUT_F], F8, tag=f"w8_{it2}",
                                name=f"w8_{it2}")
                nc.sync.dma_start(out=w8, in_=wT8_ap[it2])
                w8_tiles.append(w8)

            # ---- per-a bias tiles for Derivative_Erf: -g_a/h ----
            bias_tiles = []
            for a in range(GRID_SIZE):
                bt_ = misc.tile([P, 1], F32, tag=f"bias{a}", name=f"bias{a}")
                nc.vector.memset(bt_, float(-grid[a] / H))
                bias_tiles.append(bt_)

            # dummy activation: hoists the one-time ACT_TABLE_LOAD early
            scr = misc.tile([P, 1], F32, tag="scr", name="scr")
            nc.scalar.activation(out=scr, in_=bias_tiles[0],
                                 func=AF.Derivative_Erf,
                                 bias=bias_tiles[0], scale=1.0 / H)

            # ---- PE warmup train (HAM clock-gate): bridges the PE until the
            # real stream starts so HAM un-throttles before real MMs run
            wp = psumB_pool.tile([P, B_TILE], F32, tag="psB", name="warm_ps")
            for i in range(N_WARMUP_MM):
                nc.tensor.matmul(wp, wa, wb_, start=(i == 0),
                                 stop=(i == N_WARMUP_MM - 1))

            ot_tiles = {}
            pair_tiles = {}

            def host_cluster(btc, o):
                """Run btc's 6 DR matmuls for o-block o, combine into its out
                tile, and DMA it. Runs inside b-tile btc+1's window."""
                osl = slice(o * P, (o + 1) * P)
                psB = psumB_pool.tile([P, B_TILE], F32, tag="psB",
                                      name=f"psB{btc}_{o}")
                for itp in range(I_TILES):
                    nc.tensor.matmul(
                        psB,
                        w8_tiles[itp][:, :, osl],
                        pair_tiles[(btc, itp)],
                        start=(itp == 0),
                        stop=(itp == I_TILES - 1),
                        perf_mode=DR,
                    )
                tmp = tmppool.tile([P, B_TILE], F32, tag="tmp",
                                   name=f"tmp{btc}_{o}")
                nc.vector.tensor_scalar_mul(tmp, psB, 1.0 / W8_SCALE)
                ot = ot_tiles[(btc, o)]
                nc.vector.tensor_tensor(ot, ot, tmp, ADD)
                bsl0 = slice(btc * B_TILE, (btc + 1) * B_TILE)
                nc.gpsimd.dma_start(out=outT_ap[osl, bsl0], in_=ot)

            # ---- main loop ----
            for bt in range(N_BTILES):
                bsl = slice(bt * B_TILE, (bt + 1) * B_TILE)
                last = bt == N_BTILES - 1

                # phi production: 6 bf16 planes + 1 fp8 pair tile per octave
                phis = []
                for it in range(I_TILES):
                    for a in A_BF:
                        if last:
                            ph = phi3pool.tile([P, B_TILE], BF16,
                                               tag=f"phi3_{it}_{a}",
                                               name=f"phi3_{it}_{a}")
                        else:
                            ph = phipool.tile([P, B_TILE], BF16, tag="phi",
                                              name=f"phi{bt}_{it}_{a}")
                        nc.scalar.activation(out=ph, in_=x_tiles[(bt, it)],
                                             func=AF.Derivative_Erf,
                                             bias=bias_tiles[a], scale=1.0 / H)
                        phis.append(ph)
                    if last:
                        pr = phi3pool.tile([P, 2, B_TILE], F8,
                                           tag=f"pair3_{it}",
                                           name=f"pair3_{it}")
                    else:
                        pr = pairpool.tile([P, 2, B_TILE], F8, tag="pair",
                                           name=f"pair{bt}_{it}")
                    for j, a in enumerate(A_F8):
                        nc.scalar.activation(out=pr[:, j, :],
                                             in_=x_tiles[(bt, it)],
                                             func=AF.Derivative_Erf,
                                             bias=bias_tiles[a], scale=1.0 / H)
                    pair_tiles[(bt, it)] = pr

                if not last:
                    psums = []
                    for o in range(O_TILES):
                        ps = psum_pool.tile([P, B_TILE], F32, tag="ps",
                                            name=f"ps{bt}_{o}")
                        psums.append(ps)
                    for k in range(K_BF):
                        it = k // len(A_BF)
                        for o in range(O_TILES):
                            nc.tensor.matmul(
                                psums[o],
                                w_tiles[k][:, o * P:(o + 1) * P],
                                phis[k],
                                start=(k == 0),
                                stop=(k == K_BF - 1),
                            )
                        # host the previous b-tile's DR cluster for o=it
                        # right after octave it's bf16 matmuls
                        if bt > 0 and k % len(A_BF) == len(A_BF) - 1:
                            host_cluster(bt - 1, it)
                    for o in range(O_TILES):
                        ot = opool.tile([P, B_TILE], F32, tag="out",
                                        name=f"out{bt}_{o}")
                        nc.vector.tensor_copy(ot, psums[o])
                        ot_tiles[(bt, o)] = ot
                else:
                    # o-major passes; each hosts bt2's DR cluster + its own
                    for o in range(O_TILES):
                        osl = slice(o * P, (o + 1) * P)
                        host_cluster(N_BTILES - 2, o)

                        psB3 = psumB_pool.tile([P, B_TILE], F32, tag="psB",
                                               name=f"psB3_{o}")
                        for itp in range(I_TILES):
                            nc.tensor.matmul(
                                psB3,
                                w8_tiles[itp][:, :, osl],
                                pair_tiles[(bt, itp)],
                                start=(itp == 0),
                                stop=(itp == I_TILES - 1),
                                perf_mode=DR,
                            )

                        if o == O_TILES - 1:
                            # final pass: two half-psum accumulations so the
                            # first half's drain + DMA overlap the second
                            # half's matmuls; DR result scaled once up front
                            tmp = tmppool.tile([P, B_TILE], F32, tag="tmp",
                                               name=f"tmp3_{o}")
                            nc.vector.tensor_scalar_mul(tmp, psB3,
                                                        1.0 / W8_SCALE)
                            ot = opool.tile([P, B_TILE], F32, tag="out",
                                            name=f"out3_{o}")
                            hb = B_TILE // 2
                            b0 = bt * B_TILE
                            engs = [nc.sync, nc.scalar]
                            for half in range(2):
                                hsl = slice(half * hb, (half + 1) * hb)
                                psh = psum_pool.tile([P, hb], F32, tag="ps",
                                                     name=f"ps3_{o}_{half}")
                                for k in range(K_BF):
                                    nc.tensor.matmul(
                                        psh,
                                        w_tiles[k][:, osl],
                                        phis[k][:, hsl],
                                        start=(k == 0),
                                        stop=(k == K_BF - 1),
                                    )
                                nc.vector.tensor_copy(ot[:, hsl], psh)
                                nc.vector.tensor_tensor(ot[:, hsl],
                                                        ot[:, hsl],
                                                        tmp[:, hsl], ADD)
                                engs[half].dma_start(
                                    out=outT_ap[osl,
                                                b0 + half * hb:
                                                b0 + (half + 1) * hb],
                                    in_=ot[:, hsl])
                        else:
                            ps = psum_pool.tile([P, B_TILE], F32, tag="ps",
                                                name=f"ps3_{o}")
                            for k in range(K_BF):
                                nc.tensor.matmul(
                                    ps,
                                    w_tiles[k][:, osl],
                                    phis[k],
                                    start=(k == 0),
                                    stop=(k == K_BF - 1),
                                )
                            ot = opool.tile([P, B_TILE], F32, tag="out",
                                            name=f"out3_{o}")
                            nc.vector.tensor_copy(ot, ps)
                            tmp = tmppool.tile([P, B_TILE], F32, tag="tmp",
                                               name=f"tmp3_{o}")
                            nc.vector.tensor_scalar_mul(tmp, psB3,
                                                        1.0 / W8_SCALE)
                            nc.vector.tensor_tensor(ot, ot, tmp, ADD)
                            nc.sync.dma_start(out=outT_ap[osl, bsl], in_=ot)

    nc.compile()
    return nc


_NC_CACHE = {}


def _get_nc():
    if "nc" not in _NC_CACHE:
        _NC_CACHE["nc"] = _build_nc()
    return _NC_CACHE["nc"]


def _fold_weights(c, w_s, w_b):
    """Host fold: cw[a,o,i] = SP2*(c*w_s + BETA[a]*w_b); bf16 central planes
    as [K_BF, P, OUT_F], fp8 edge pairs as [I_TILES, P, 2, OUT_F]*256."""
    beta = np.asarray(BETA, dtype=np.float32)[:, None, None]
    cw = SP2 * (c * w_s[None, :, :] + beta * w_b[None, :, :])  # [a, o, i]
    W = np.ascontiguousarray(cw.transpose(0, 2, 1))            # [a, i, o]
    Wk = np.empty((K_BF, P, OUT_F), np.float32)
    for it in range(I_TILES):
        for ja, a in enumerate(A_BF):
            Wk[it * len(A_BF) + ja] = W[a, it * P:(it + 1) * P, :]
    W8 = np.empty((I_TILES, P, 2, OUT_F), np.float32)
    for it in range(I_TILES):
        for j, a in enumerate(A_F8):
            W8[it, :, j, :] = W[a, it * P:(it + 1) * P, :] * W8_SCALE
    return (Wk.astype(ml_dtypes.bfloat16),
            W8.astype(ml_dtypes.float8_e4m3))


def kernel(x, w_b, w_s, c):
    x = np.ascontiguousarray(np.asarray(x, dtype=np.float32))
    w_b = np.asarray(w_b, dtype=np.float32)
    w_s = np.asarray(w_s, dtype=np.float32)
    c = np.asarray(c, dtype=np.float32)

    xT = np.ascontiguousarray(x.T).astype(ml_dtypes.bfloat16)  # [IN_F, BATCH]
    wT, wT8 = _fold_weights(c, w_s, w_b)

    in_maps = []
    for ci in range(N_CORES):
        in_maps.append({
            "xT": np.ascontiguousarray(xT[:, ci * B_SHARD:(ci + 1) * B_SHARD]),
            "wT": wT,
            "wT8": wT8,
        })

    res = run_bass_kernel_spmd(_get_nc(), in_maps, core_ids=list(range(N_CORES)))
    outT = np.concatenate([r["outT"] for r in res.results], axis=1)
    return np.ascontiguousarray(outT.T).astype(np.float32, copy=False)


if __name__ == "__main__":
    rng = np.random.default_rng(0)
    x = rng.standard_normal((BATCH, IN_F), dtype=np.float32)
    w_b = rng.standard_normal((OUT_F, IN_F), dtype=np.float32) * 1e-3
    w_s = np.ones((OUT_F, IN_F), dtype=np.float32)
    c = (rng.standard_normal((GRID_SIZE, OUT_F, IN_F)) * 1e-3).astype(np.float32)
    out = kernel(x, w_b, w_s, c)
    print(out.shape, out.dtype)


# revision 18
# speedup vs baseline: 1.0808x; 1.0015x over previous
"""LinearKAN Trainium2 kernel — bf16 central planes + fp8 DoubleRow edge planes.

Math (per reference):
    phi[b,a,i] = exp(-((x[b,i] - g_a)/h)^2)     g = linspace(-2, 2, 8), h = 4/7
    out[b,o]   = sum_{a,i} phi[b,a,i]*(c[a,o,i]*w_s[o,i]) + sum_i silu(x[b,i])*w_b[o,i]

Precision split: the EDGE grid planes carry little phi energy under
x~N(0,1), so they go to fp8-e4m3 DoubleRow pairs while central planes stay
bf16. Planes (0,7) are fp8 for ALL i-tiles; planes (1,6) additionally for
i-tiles ITS16=(0,1,2). Per (bt,o) that is 30 bf16 matmuls + 9 DR matmuls
(one 9-deep accumulation group) instead of 48 bf16 — 19% fewer PE slots.
Measured rel_l2 1.761e-2 / absmax-scale 1.91e-2 vs the 2e-2 gate
(deterministic: fixed seed, fixed schedule).

Structure per core (batch-sharded, B_SHARD=2048 over 4 b-tiles of 512):
  - bf16 part: 36 k-tiles (it x a=1..6), k-major, 6 psum banks (o=0..5),
    phi via one ACT op each (Derivative_Erf = 2/sqrt(pi) exp(-z^2), the
    sqrt(pi)/2 folded into W).
  - fp8 part: per (bt, it) pair tiles [128, 2, 512] f8e4 hold (phi_a0,
    phi_a7) and, for ITS16 i-tiles, (phi_a1, phi_a6); weights [128, 2,
    768] f8e4 scaled by 256. Per (bt, o) the 9 DR matmuls accumulate in a
    SEPARATE psum bank; combined as ot += psB * 2^-8 on the DVE at drain.
  - PSUM budget forces the DR clusters of b-tile N to run INSIDE b-tile
    N+1's k-loop (6 bf16-psums + 2 rotating DR-psums = 8 banks). The last
    b-tile is o-major, hosting bt2's clusters and its own per pass.
  - silu residual folded into ALL planes via the N(0,1)-weighted lsq fit
    silu ~= sum_a BETA[a] phi_a.

Schedule notes (measured on HW):
  - DR matmuls pace at ~215ns (1024 fp8 cols at 2/cycle) when CLUSTERED;
    alternating bf16<->DR costs ~190ns mode-switch each way, so clusters
    of 6 beat 1:1 interleaving.
  - x is bf16 end-to-end (+5e-4 rel err): halves x DMA traffic so the
    sync queue can feed W at the warm MM stream's 151 GB/s from t~12us.
  - all x tiles issue in the first ~35us (late x issues head-of-line
    block the ACT queue via scheduler-hoisted future-bt phi ACTIVATEs).
  - 11-matmul warmup train on gpsimd-memset tiles bridges the PE until
    the real stream so the HAM clock-gate opens (2.4GHz) before it.
  - out tiles DMA via the gpsimd queue (256KB f32 tiles on sync would
    delay the next octaves' x/W arrivals at b-tile boundaries).
"""

import ml_dtypes
import numpy as np

import concourse.bacc as bacc
import concourse.tile as tile
from concourse import mybir
from concourse.bass_utils import run_bass_kernel_spmd

N_CORES = 8
BATCH, IN_F, OUT_F = 16384, 768, 768
B_SHARD = BATCH // N_CORES          # 2048
GRID_SIZE, GRID_LO, GRID_HI = 8, -2.0, 2.0
H = (GRID_HI - GRID_LO) / (GRID_SIZE - 1)
P = 128
I_TILES = IN_F // P                 # 6
O_TILES = OUT_F // P                # 6
A_F8 = (0, 7)                       # fp8 DoubleRow pair (all i-tiles)
A_F8B = (1, 6)                      # second fp8 pair (ITS16 i-tiles only)
ITS16 = (0, 1, 2)                   # i-tiles whose planes 1,6 are fp8 too


def a_bf(it):
    return [2, 3, 4, 5] if it in ITS16 else [1, 2, 3, 4, 5, 6]


KS = [(it, a) for it in range(I_TILES) for a in a_bf(it)]
K_BF = len(KS)                      # 30 bf16 k-tiles
OCT_END = [max(i for i, (it2, _) in enumerate(KS) if it2 == it)
           for it in range(I_TILES)]
B_TILE = 512
N_BTILES = B_SHARD // B_TILE        # 4
W8_SCALE = 256.0

F32 = mybir.dt.float32
BF16 = mybir.dt.bfloat16
F8 = mybir.dt.float8e4
AF = mybir.ActivationFunctionType
DR = mybir.MatmulPerfMode.DoubleRow
ADD = mybir.AluOpType.add
SP2 = float(np.sqrt(np.pi) / 2.0)

N_WARMUP_MM = 11


def _silu_fit():
    X = np.linspace(-5.6, 5.6, 4481)
    W = np.exp(-X * X / 2.0)
    SW = np.sqrt(W / W.sum())
    grid = np.linspace(GRID_LO, GRID_HI, GRID_SIZE)
    cols = [np.exp(-(((X - g) / H) ** 2)) for g in grid]
    A = (np.array(cols) * SW[None, :]).T
    b = (X / (1.0 + np.exp(-X))) * SW
    coef, *_ = np.linalg.lstsq(A, b, rcond=None)
    return [float(v) for v in coef]


BETA = _silu_fit()


def _build_nc():
    nc = bacc.Bacc(None, target_bir_lowering=False, debug=False)

    xT = nc.dram_tensor("xT", [IN_F, B_SHARD], BF16, kind="ExternalInput")
    wT = nc.dram_tensor("wT", [K_BF, P, OUT_F], BF16, kind="ExternalInput")
    wT8 = nc.dram_tensor("wT8", [I_TILES, P, 2, OUT_F], F8,
                         kind="ExternalInput")
    wT8b = nc.dram_tensor("wT8b", [len(ITS16), P, 2, OUT_F], F8,
                          kind="ExternalInput")
    outT = nc.dram_tensor("outT", [OUT_F, B_SHARD], F32, kind="ExternalOutput")

    xT_ap = xT.ap()
    wT_ap = wT.ap()
    wT8_ap = wT8.ap()
    wT8b_ap = wT8b.ap()
    outT_ap = outT.ap()

    grid = np.linspace(GRID_LO, GRID_HI, GRID_SIZE, dtype=np.float64)

    with tile.TileContext(nc) as tc:
        with (
            tc.tile_pool(name="wpool", bufs=1) as wpool,
            tc.tile_pool(name="misc", bufs=1) as misc,
            tc.tile_pool(name="xpool", bufs=24) as xpool,
            tc.tile_pool(name="phipool", bufs=20) as phipool,
            tc.tile_pool(name="pairpool", bufs=19) as pairpool,
            tc.tile_pool(name="phi3pool", bufs=1) as phi3pool,
            tc.tile_pool(name="opool", bufs=12) as opool,
            tc.tile_pool(name="tmppool", bufs=3) as tmppool,
            tc.tile_pool(name="psum", bufs=6, space="PSUM") as psum_pool,
            tc.tile_pool(name="psumB", bufs=2, space="PSUM") as psumB_pool,
        ):
            # ---- PE warmup tiles first: gpsimd memsets run earliest
            wa = misc.tile([P, P], BF16, tag="warm_a", name="warm_a")
            nc.gpsimd.memset(wa, 0.0)
            wb_ = misc.tile([P, B_TILE], BF16, tag="warm_b", name="warm_b")
            nc.gpsimd.memset(wb_, 0.0)

            # ---- x00 split into partition-row quarters across 4 DMA queues
            x_tiles = {}
            xt = xpool.tile([P, B_TILE], BF16, tag="x", name="x0_0")
            qp = P // 4
            qeng = [nc.sync, nc.scalar, nc.scalar, nc.gpsimd]
            for q in range(4):
                qeng[q].dma_start(out=xt[q * qp:(q + 1) * qp, :],
                                  in_=xT_ap[q * qp:(q + 1) * qp, 0:B_TILE])
            x_tiles[(0, 0)] = xt

            w_tiles = [None] * K_BF

            def w_load(k):
                wt = wpool.tile([P, OUT_F], BF16, tag=f"w{k}", name=f"w{k}")
                nc.sync.dma_start(out=wt, in_=wT_ap[k])
                w_tiles[k] = wt

            def x_load(bt, it):
                bsl0 = slice(bt * B_TILE, (bt + 1) * B_TILE)
                xt2 = xpool.tile([P, B_TILE], BF16, tag="x", name=f"x{bt}_{it}")
                nc.sync.dma_start(out=xt2,
                                  in_=xT_ap[it * P:(it + 1) * P, bsl0])
                x_tiles[(bt, it)] = xt2

            # sync FIFO: W back-to-back with x tiles threaded in early enough
            # that every x lands by ~35us (late x issues head-of-line block
            # the ACT queue via scheduler-hoisted future-bt phi ACTIVATEs)
            kq = 0
            for it2 in range(1, I_TILES):
                w_load(kq); w_load(kq + 1)
                kq += 2
                x_load(0, it2)
            rest_x = [(bt, it) for bt in range(1, N_BTILES)
                      for it in range(I_TILES)]
            for bt2, it2 in rest_x:
                if kq < K_BF:
                    w_load(kq)
                    kq += 1
                x_load(bt2, it2)
            for k in range(kq, K_BF):
                w_load(k)
            # fp8 weight pairs: consumed from bt1 onward (~75us) — load last
            w8_tiles = []
            for it2 in range(I_TILES):
                w8 = wpool.tile([P, 2, OUT_F], F8, tag=f"w8_{it2}",
                                name=f"w8_{it2}")
                nc.sync.dma_start(out=w8, in_=wT8_ap[it2])
                w8_tiles.append(w8)
            w8b_tiles = []
            for j in range(len(ITS16)):
                w8b = wpool.tile([P, 2, OUT_F], F8, tag=f"w8b_{j}",
                                 name=f"w8b_{j}")
                nc.sync.dma_start(out=w8b, in_=wT8b_ap[j])
                w8b_tiles.append(w8b)

            # ---- per-a bias tiles for Derivative_Erf: -g_a/h ----
            bias_tiles = []
            for a in range(GRID_SIZE):
                bt_ = misc.tile([P, 1], F32, tag=f"bias{a}", name=f"bias{a}")
                nc.vector.memset(bt_, float(-grid[a] / H))
                bias_tiles.append(bt_)

            # dummy activation: hoists the one-time ACT_TABLE_LOAD early
            scr = misc.tile([P, 1], F32, tag="scr", name="scr")
            nc.scalar.activation(out=scr, in_=bias_tiles[0],
                                 func=AF.Derivative_Erf,
                                 bias=bias_tiles[0], scale=1.0 / H)

            # ---- PE warmup train (HAM clock-gate): bridges the PE until the
            # real stream starts so HAM un-throttles before real MMs run
            wp = psumB_pool.tile([P, B_TILE], F32, tag="psB", name="warm_ps")
            for i in range(N_WARMUP_MM):
                nc.tensor.matmul(wp, wa, wb_, start=(i == 0),
                                 stop=(i == N_WARMUP_MM - 1))

            ot_tiles = {}
            pair_tiles = {}

            def dr_ops(btc):
                ops = [(w8_tiles[itp], pair_tiles[(btc, 'a', itp)])
                       for itp in range(I_TILES)]
                ops += [(w8b_tiles[j], pair_tiles[(btc, 'b', it2)])
                        for j, it2 in enumerate(ITS16)]
                return ops

            def host_cluster(btc, o):
                """Run btc's 9 DR matmuls for o-block o (one accumulation
                group), combine into its out tile, and DMA it. Runs inside
                b-tile btc+1's window."""
                osl = slice(o * P, (o + 1) * P)
                psB = psumB_pool.tile([P, B_TILE], F32, tag="psB",
                                      name=f"psB{btc}_{o}")
                ops = dr_ops(btc)
                for j, (w8t, pr) in enumerate(ops):
                    nc.tensor.matmul(
                        psB,
                        w8t[:, :, osl],
                        pr,
                        start=(j == 0),
                        stop=(j == len(ops) - 1),
                        perf_mode=DR,
                    )
                tmp = tmppool.tile([P, B_TILE], F32, tag="tmp",
                                   name=f"tmp{btc}_{o}")
                nc.vector.tensor_scalar_mul(tmp, psB, 1.0 / W8_SCALE)
                ot = ot_tiles[(btc, o)]
                nc.vector.tensor_tensor(ot, ot, tmp, ADD)
                bsl0 = slice(btc * B_TILE, (btc + 1) * B_TILE)
                nc.gpsimd.dma_start(out=outT_ap[osl, bsl0], in_=ot)

            # ---- main loop ----
            for bt in range(N_BTILES):
                bsl = slice(bt * B_TILE, (bt + 1) * B_TILE)
                last = bt == N_BTILES - 1

                # phi production: 6 bf16 planes + 1 fp8 pair tile per octave
                phis = []
                for it in range(I_TILES):
                    for a in a_bf(it):
                        if last:
                            ph = phi3pool.tile([P, B_TILE], BF16,
                                               tag=f"phi3_{it}_{a}",
                                               name=f"phi3_{it}_{a}")
                        else:
                            ph = phipool.tile([P, B_TILE], BF16, tag="phi",
                                              name=f"phi{bt}_{it}_{a}")
                        nc.scalar.activation(out=ph, in_=x_tiles[(bt, it)],
                                             func=AF.Derivative_Erf,
                                             bias=bias_tiles[a], scale=1.0 / H)
                        phis.append(ph)
                    kinds = [('a', A_F8)]
                    if it in ITS16:
                        kinds.append(('b', A_F8B))
                    for kind, planes in kinds:
                        if last:
                            pr = phi3pool.tile([P, 2, B_TILE], F8,
                                               tag=f"pair3{kind}_{it}",
                                               name=f"pair3{kind}_{it}")
                        else:
                            pr = pairpool.tile([P, 2, B_TILE], F8, tag="pair",
                                               name=f"pair{bt}{kind}_{it}")
                        for j, a in enumerate(planes):
                            nc.scalar.activation(out=pr[:, j, :],
                                                 in_=x_tiles[(bt, it)],
                                                 func=AF.Derivative_Erf,
                                                 bias=bias_tiles[a],
                                                 scale=1.0 / H)
                        pair_tiles[(bt, kind, it)] = pr

                if not last:
                    psums = []
                    for o in range(O_TILES):
                        ps = psum_pool.tile([P, B_TILE], F32, tag="ps",
                                            name=f"ps{bt}_{o}")
                        psums.append(ps)
                    for k in range(K_BF):
                        it = KS[k][0]
                        for o in range(O_TILES):
                            nc.tensor.matmul(
                                psums[o],
                                w_tiles[k][:, o * P:(o + 1) * P],
                                phis[k],
                                start=(k == 0),
                                stop=(k == K_BF - 1),
                            )
                        # host the previous b-tile's DR cluster for o=it
                        # right after octave it's bf16 matmuls
                        if bt > 0 and k == OCT_END[it]:
                            host_cluster(bt - 1, it)
                    for o in range(O_TILES):
                        ot = opool.tile([P, B_TILE], F32, tag="out",
                                        name=f"out{bt}_{o}")
                        nc.vector.tensor_copy(ot, psums[o])
                        ot_tiles[(bt, o)] = ot
                else:
                    # o-major passes; each hosts bt2's DR cluster + its own
                    for o in range(O_TILES):
                        osl = slice(o * P, (o + 1) * P)
                        host_cluster(N_BTILES - 2, o)

                        psB3 = psumB_pool.tile([P, B_TILE], F32, tag="psB",
                                               name=f"psB3_{o}")
                        ops3 = dr_ops(bt)
                        for j, (w8t, pr) in enumerate(ops3):
                            nc.tensor.matmul(
                                psB3,
                                w8t[:, :, osl],
                                pr,
                                start=(j == 0),
                                stop=(j == len(ops3) - 1),
                                perf_mode=DR,
                            )

                        if o == O_TILES - 1:
                            # final pass: two half-psum accumulations so the
                            # first half's drain + DMA overlap the second
                            # half's matmuls; DR result scaled once up front
                            tmp = tmppool.tile([P, B_TILE], F32, tag="tmp",
                                               name=f"tmp3_{o}")
                            nc.vector.tensor_scalar_mul(tmp, psB3,
                                                        1.0 / W8_SCALE)
                            ot = opool.tile([P, B_TILE], F32, tag="out",
                                            name=f"out3_{o}")
                            hb = B_TILE // 2
                            b0 = bt * B_TILE
                            engs = [nc.sync, nc.scalar]
                            for half in range(2):
                                hsl = slice(half * hb, (half + 1) * hb)
                                psh = psum_pool.tile([P, hb], F32, tag="ps",
                                                     name=f"ps3_{o}_{half}")
                                for k in range(K_BF):
                                    nc.tensor.matmul(
                                        psh,
                                        w_tiles[k][:, osl],
                                        phis[k][:, hsl],
                                        start=(k == 0),
                                        stop=(k == K_BF - 1),
                                    )
                                nc.vector.tensor_copy(ot[:, hsl], psh)
                                nc.vector.tensor_tensor(ot[:, hsl],
                                                        ot[:, hsl],
                                                        tmp[:, hsl], ADD)
                                engs[half].dma_start(
                                    out=outT_ap[osl,
                                                b0 + half * hb:
                                                b0 + (half + 1) * hb],
                                    in_=ot[:, hsl])
                        else:
                            ps = psum_pool.tile([P, B_TILE], F32, tag="ps",
                                                name=f"ps3_{o}")
                            for k in range(K_BF):
                                nc.tensor.matmul(
                                    ps,
                                    w_tiles[k][:, osl],
                                    phis[k],
                                    start=(k == 0),
                                    stop=(k == K_BF - 1),
                                )
                            ot = opool.tile([P, B_TILE], F32, tag="out",
                                            name=f"out3_{o}")
                            nc.vector.tensor_copy(ot, ps)
                            tmp = tmppool.tile([P, B_TILE], F32, tag="tmp",
                                               name=f"tmp3_{o}")
                            nc.vector.tensor_scalar_mul(tmp, psB3,
                                                        1.0 / W8_SCALE)
                            nc.vector.tensor_tensor(ot, ot, tmp, ADD)
                            nc.sync.dma_start(out=outT_ap[osl, bsl], in_=ot)

    nc.compile()
    return nc


_NC_CACHE = {}


def _get_nc():
    if "nc" not in _NC_CACHE:
        _NC_CACHE["nc"] = _build_nc()
    return _NC_CACHE["nc"]


def _fold_weights(c, w_s, w_b):
    """Host fold: cw[a,o,i] = SP2*(c*w_s + BETA[a]*w_b); bf16 central planes
    as [K_BF, P, OUT_F], fp8 edge pairs as [I_TILES, P, 2, OUT_F]*256."""
    beta = np.asarray(BETA, dtype=np.float32)[:, None, None]
    cw = SP2 * (c * w_s[None, :, :] + beta * w_b[None, :, :])  # [a, o, i]
    W = np.ascontiguousarray(cw.transpose(0, 2, 1))            # [a, i, o]
    Wk = np.empty((K_BF, P, OUT_F), np.float32)
    for k, (it, a) in enumerate(KS):
        Wk[k] = W[a, it * P:(it + 1) * P, :]
    W8 = np.empty((I_TILES, P, 2, OUT_F), np.float32)
    for it in range(I_TILES):
        for j, a in enumerate(A_F8):
            W8[it, :, j, :] = W[a, it * P:(it + 1) * P, :] * W8_SCALE
    W8b = np.empty((len(ITS16), P, 2, OUT_F), np.float32)
    for jt, it in enumerate(ITS16):
        for j, a in enumerate(A_F8B):
            W8b[jt, :, j, :] = W[a, it * P:(it + 1) * P, :] * W8_SCALE
    return (Wk.astype(ml_dtypes.bfloat16),
            W8.astype(ml_dtypes.float8_e4m3),
            W8b.astype(ml_dtypes.float8_e4m3))


def kernel(x, w_b, w_s, c):
    x = np.ascontiguousarray(np.asarray(x, dtype=np.float32))
    w_b = np.asarray(w_b, dtype=np.float32)
    w_s = np.asarray(w_s, dtype=np.float32)
    c = np.asarray(c, dtype=np.float32)

    xT = np.ascontiguousarray(x.T).astype(ml_dtypes.bfloat16)  # [IN_F, BATCH]
    wT, wT8, wT8b = _fold_weights(c, w_s, w_b)

    in_maps = []
    for ci in range(N_CORES):
        in_maps.append({
            "xT": np.ascontiguousarray(xT[:, ci * B_SHARD:(ci + 1) * B_SHARD]),
            "wT": wT,
            "wT8": wT8,
            "wT8b": wT8b,
        })

    res = run_bass_kernel_spmd(_get_nc(), in_maps, core_ids=list(range(N_CORES)))
    outT = np.concatenate([r["outT"] for r in res.results], axis=1)
    return np.ascontiguousarray(outT.T).astype(np.float32, copy=False)


if __name__ == "__main__":
    rng = np.random.default_rng(0)
    x = rng.standard_normal((BATCH, IN_F), dtype=np.float32)
    w_b = rng.standard_normal((OUT_F, IN_F), dtype=np.float32) * 1e-3
    w_s = np.ones((OUT_F, IN_F), dtype=np.float32)
    c = (rng.standard_normal((GRID_SIZE, OUT_F, IN_F)) * 1e-3).astype(np.float32)
    out = kernel(x, w_b, w_s, c)
    print(out.shape, out.dtype)
